# revision 34
# baseline (speedup 1.0000x reference)
"""Trainium2 Bass kernel for nn_AdaptedEntropyModel (vq_codebook).

reference:
    r = x - means
    symbols = argmin_i |codebook[i] - r|   (ties -> left / lower index)
    y_hat   = codebook[symbols] + means

Algorithm (exact up to f32 boundary rounding):
  with sorted codebook c_i, midpoints m_i = (c_i + c_{i+1})/2 and
  deltas D_i = c_{i+1} - c_i (i = 0..62):
      b_i     = [r > m_i]
      symbols = sum_i b_i
      y_hat   = c_0 + sum_i D_i b_i + means

Both sums are packed into ONE fused accumulator per element:
      z = sum_i W_i * s_i,   W_i = (D_i + K)/2,   s_i = sign(r - m_i)
  so  z + C = K*symbols + y_off   (C = sum_i W_i, y_off = sum_i D_i b_i,
                                   0 <= y_off << K = 128)
      symbols = round((z + C)/K)        (f32->i32 convert rounds nearest)
      y_hat   = (z + C - K*symbols) + c_0 + means

The signs are produced on the otherwise-idle scalar engine (ACT) via
sign(fma(r, 3, beta_i)); beta_i ~ -3*m_i is nudged so its f32 mantissa is
not divisible by 3, which makes 3*r + beta_i != 0 for EVERY f32 r - the
hardware affine is a true fused multiply-add, so sign() can never return
0 and each element lands cleanly on one side (verified on silicon). The
DVE then needs just ONE fused scalar_tensor_tensor (mult, add) per level
instead of separate symbol/value chains - it is the critical path at
~2.1 us per [128 x 2048] level.

Sharding: pure data parallel over batch; each of the 8 cores gets 4
consecutive batches (contiguous 3,145,728 f32), viewed as [128, 24576].
x and means are interleaved host-side into one [128, 2*FREE] input so
each tile is loaded by a single DMA (single wait semaphore - the V3 ISA
allows only one sync wait per instruction). The codebook-derived
constants are baked per build; kernel() re-builds if the codebook
changes.
"""

import sys

import numpy as np

if "/opt/trn_rl_repo" not in sys.path:
    sys.path.insert(0, "/opt/trn_rl_repo")

B, C, H, W = 32, 192, 64, 64
L = 64
N_CORES = 8
TOT = B * C * H * W            # 25_165_824
PER_CORE = TOT // N_CORES      # 3_145_728
P = 128
FREE = PER_CORE // P           # 24576
TILE_F = 2048
N_TILES = FREE // TILE_F       # 12
K_ENC = 128.0                  # symbol step in the packed accumulator
Z_SPLIT = 2                    # independent accumulator chains per tile
SGN_BUFS = 8                   # ACT sign-plane run-ahead buffers
REPEAT = 1                     # whole-kernel repetitions (timing slope only)
ACT_DECODE = False             # run the two decode converts on ACT
SYM_I8 = True                  # device writes int8 symbols; host casts to int32
INP_BUFS = 3
PIPE_MID = 56                  # level index at which the next tile's load+sub is emitted
OUTP_BUFS = 2


def _coprime3_beta(m):
    """f32 beta ~ -3*m whose integer mantissa is not divisible by 3, so
    fma(r, 3, beta) is never exactly 0 for any f32 r."""
    b = np.float32(-3.0 * m)
    if b == 0.0 or not np.isfinite(b):
        b = np.float32(1e-30)
    for _ in range(4):
        mant = int(np.abs(b).view(np.uint32) & 0x7FFFFF) | 0x800000
        if mant % 3 != 0:
            return float(b)
        b = np.nextafter(b, np.float32(np.sign(b) * np.float32(1e38)),
                         dtype=np.float32)
    return float(b)


def _build(weights, betas, dec_scale, dec_bias, y_bias):
    """Build the per-core SPMD Bass program.

    weights[i] = (D_i + K)/2 (stt scalar per level)
    betas[i]   = ACT bias for level i (threshold -beta/3)
    dec_scale  = 1/K, dec_bias = C/K      (symbol decode ts)
    y_bias     = C + c_0                  (value decode stt)
    """
    from contextlib import ExitStack

    import concourse.bass as bass
    import concourse.tile as tile
    from concourse import bacc, mybir

    f32 = mybir.dt.float32
    i32 = mybir.dt.int32
    Alu = mybir.AluOpType
    Act = mybir.ActivationFunctionType

    nc = bacc.Bacc(
        "TRN2",
        target_bir_lowering=False,
        debug=False,
        num_devices=N_CORES,
    )
    # row p = [x row | means row]: one DMA per tile feeds both halves
    xm = nc.dram_tensor("xm", [P, 2 * FREE], f32, kind="ExternalInput")
    xm_r = xm.rearrange("p (h q) -> p h q", h=2)
    # per-partition replicated constants: column i holds betas[i]
    nmid = nc.dram_tensor("nmid", [P, L], f32, kind="ExternalInput")
    i8 = mybir.dt.int8
    sym_out = nc.dram_tensor("sym", [P, FREE], i8 if SYM_I8 else i32,
                             kind="ExternalOutput")
    y_out = nc.dram_tensor("y", [P, FREE], f32, kind="ExternalOutput")

    S = Z_SPLIT
    with tile.TileContext(nc) as tc, ExitStack() as ctx:
        inp = ctx.enter_context(tc.tile_pool(name="inp", bufs=INP_BUFS))
        work = ctx.enter_context(tc.tile_pool(name="work", bufs=1))
        sgn = ctx.enter_context(tc.tile_pool(name="sgn", bufs=SGN_BUFS))
        outp = ctx.enter_context(tc.tile_pool(name="outp", bufs=OUTP_BUFS))
        cst = ctx.enter_context(tc.tile_pool(name="cst", bufs=1))

        nmt = cst.tile([P, L], f32, tag="nmt")
        nc.sync.dma_start(nmt[:], nmid[:])

        steps = REPEAT * N_TILES

        def emit_load_sub(k):
            # load tile k's interleaved input and compute r = x - means;
            # called mid-way through tile k-1's chain so the scalar engine
            # can pre-generate tile k's sign planes (kills the tile-boundary
            # bubble where DVE would wait on the first signs).
            sl_k = bass.ts(k % N_TILES, TILE_F)
            txm = inp.tile([P, 2 * TILE_F], f32, tag="txm", name=f"txm_{k}")
            nc.sync.dma_start(
                txm[:].rearrange("p (h f) -> p h f", h=2), xm_r[:, :, sl_k]
            )
            r = work.tile([P, TILE_F], f32, tag=f"r{k % 2}", name=f"r_{k}")
            nc.vector.tensor_sub(r[:], txm[:, :TILE_F], txm[:, TILE_F:])
            return txm, r

        nxt = emit_load_sub(0)
        for k in range(steps):
            t = k % N_TILES
            sl = bass.ts(t, TILE_F)
            txm, r = nxt
            tm = txm[:, TILE_F:]

            # packed accumulator, S independent in-place chains:
            #   z_c += W_i * sign(3r + beta_i)   (levels round-robin)
            zs = [
                work.tile([P, TILE_F], f32, tag=f"z{c}{k % 2}",
                          name=f"z{c}_{k}")
                for c in range(S)
            ]
            for i in range(L - 1):
                si = sgn.tile([P, TILE_F], f32, tag="s")
                nc.scalar.activation(si[:], r[:], Act.Sign,
                                     bias=nmt[:, i:i + 1], scale=3.0)
                z = zs[i % S]
                if i < S:
                    nc.vector.tensor_scalar(z[:], si[:], weights[i], None,
                                            op0=Alu.mult)
                else:
                    nc.vector.scalar_tensor_tensor(
                        z[:], si[:], weights[i], z[:],
                        op0=Alu.mult, op1=Alu.add,
                    )
                if i == PIPE_MID and k + 1 < steps:
                    nxt = emit_load_sub(k + 1)
            for c in range(1, S):
                nc.vector.tensor_add(zs[0][:], zs[0][:], zs[c][:])
            cur = zs[0]

            # decode: sym = round(z/K + C/K)  (convert rounds to nearest)
            syi = outp.tile([P, TILE_F], i8 if SYM_I8 else i32, tag="syi")
            if ACT_DECODE:
                nc.scalar.activation(syi[:], cur[:], Act.Copy,
                                     bias=float(dec_bias), scale=dec_scale)
            else:
                nc.vector.tensor_scalar(syi[:], cur[:], dec_scale, dec_bias,
                                        op0=Alu.mult, op1=Alu.add)
            nc.sync.dma_start(sym_out[:, sl], syi[:])

            # y_hat = (z - K*symf) + (C + c0) + means
            sf = work.tile([P, TILE_F], f32, tag="sf")
            if ACT_DECODE:
                nc.scalar.activation(sf[:], syi[:], Act.Copy)
            else:
                nc.vector.tensor_scalar(sf[:], syi[:], 1.0, None,
                                        op0=Alu.mult)
            nc.vector.scalar_tensor_tensor(
                sf[:], sf[:], -K_ENC, cur[:], op0=Alu.mult, op1=Alu.add
            )
            yh = outp.tile([P, TILE_F], f32, tag="yh")
            nc.vector.scalar_tensor_tensor(
                yh[:], tm, y_bias, sf[:], op0=Alu.add, op1=Alu.add
            )
            nc.sync.dma_start(y_out[:, sl], yh[:])

    nc.compile()
    return nc


_cache = {}


def _get_nc(codebook):
    key = codebook.tobytes()
    if key not in _cache:
        cb = codebook.astype(np.float64)
        mids = ((cb[:-1] + cb[1:]) * 0.5).astype(np.float32).astype(np.float64)
        deltas = (cb[1:] - cb[:-1]).astype(np.float64)
        weights = [float(np.float32((d + K_ENC) * 0.5)) for d in deltas]
        betas = [_coprime3_beta(m) for m in mids]
        const = float(sum(np.float64(w) for w in weights))
        dec_scale = float(np.float32(1.0 / K_ENC))
        dec_bias = float(np.float32(const / K_ENC))
        y_bias = float(np.float32(const + cb[0]))
        nmid = np.zeros((P, L), np.float32)
        nmid[:, : L - 1] = np.float32(betas)[None, :]
        nc = _build(weights, betas, dec_scale, dec_bias, y_bias)
        _cache[key] = (nc, nmid)
    return _cache[key]


def _run(x, means, codebook, trace=False):
    from concourse.bass_utils import run_bass_kernel_spmd

    nc, nmid = _get_nc(np.asarray(codebook))

    x = np.asarray(x).reshape(N_CORES, P, FREE)
    means = np.asarray(means).reshape(N_CORES, P, FREE)
    in_maps = [
        {
            "xm": np.ascontiguousarray(np.concatenate([x[c], means[c]], axis=1)),
            "nmid": nmid,
        }
        for c in range(N_CORES)
    ]
    res = run_bass_kernel_spmd(
        nc, in_maps, core_ids=list(range(N_CORES)), trace=trace
    )
    sym = np.stack([res.results[c]["sym"] for c in range(N_CORES)])
    y = np.stack([res.results[c]["y"] for c in range(N_CORES)])
    sym = sym.reshape(B, C, H, W).astype(np.int32)
    y = y.reshape(B, C, H, W).astype(np.float32)
    return (sym, y), res


def kernel(x, means, codebook):
    (sym, y), _ = _run(x, means, codebook)
    return sym, y


# revision 36
# speedup vs baseline: 1.2051x; 1.2051x over previous
"""Trainium2 Bass kernel for nn_AdaptedEntropyModel (vq_codebook).

reference:
    r = x - means
    symbols = argmin_i |codebook[i] - r|   (ties -> left / lower index)
    y_hat   = codebook[symbols] + means

Algorithm (exact up to f32 boundary rounding):
  with sorted codebook c_i, midpoints m_i = (c_i + c_{i+1})/2 and
  deltas D_i = c_{i+1} - c_i (i = 0..62):
      b_i     = [r > m_i]
      symbols = sum_i b_i
      y_hat   = c_0 + sum_i D_i b_i + means

Both sums are packed into ONE fused accumulator per element:
      z = sum_i W_i * s_i,   W_i = (D_i + K)/2,   s_i = sign(r - m_i)
  so  z + C = K*symbols + y_off   (C = sum_i W_i, y_off = sum_i D_i b_i,
                                   0 <= y_off << K = 128)
      symbols = round((z + C)/K)        (f32->i32 convert rounds nearest)
      y_hat   = (z + C - K*symbols) + c_0 + means

The signs are produced on the otherwise-idle scalar engine (ACT) via
sign(fma(r, 3, beta_i)); beta_i ~ -3*m_i is nudged so its f32 mantissa is
not divisible by 3, which makes 3*r + beta_i != 0 for EVERY f32 r - the
hardware affine is a true fused multiply-add, so sign() can never return
0 and each element lands cleanly on one side (verified on silicon). The
DVE then needs just ONE fused scalar_tensor_tensor (mult, add) per level
instead of separate symbol/value chains - it is the critical path at
~2.1 us per [128 x 2048] level.

Sharding: pure data parallel over batch; each of the 8 cores gets 4
consecutive batches (contiguous 3,145,728 f32), viewed as [128, 24576].
x and means are interleaved host-side into one [128, 2*FREE] input so
each tile is loaded by a single DMA (single wait semaphore - the V3 ISA
allows only one sync wait per instruction). The codebook-derived
constants are baked per build; kernel() re-builds if the codebook
changes.
"""

import sys

import numpy as np

if "/opt/trn_rl_repo" not in sys.path:
    sys.path.insert(0, "/opt/trn_rl_repo")

B, C, H, W = 32, 192, 64, 64
L = 64
N_CORES = 8
TOT = B * C * H * W            # 25_165_824
PER_CORE = TOT // N_CORES      # 3_145_728
P = 128
FREE = PER_CORE // P           # 24576
TILE_F = 2048
N_TILES = FREE // TILE_F       # 12
K_ENC = 128.0                  # symbol step in the packed accumulator
Z_SPLIT = 2                    # independent accumulator chains per tile
SGN_BUFS = 8                   # ACT sign-plane run-ahead buffers
REPEAT = 1                     # whole-kernel repetitions (timing slope only)
ACT_DECODE = True              # run the two decode converts on ACT
ACT_INIT = True                # init the z chains on ACT (Copy, scale=W)
SYM_I8 = True                  # device writes int8 symbols; host casts to int32
INP_BUFS = 3
PIPE_MID = 56                  # level index at which the next tile's load+sub is emitted
OUTP_BUFS = 2


def _coprime3_beta(m):
    """f32 beta ~ -3*m whose integer mantissa is not divisible by 3, so
    fma(r, 3, beta) is never exactly 0 for any f32 r."""
    b = np.float32(-3.0 * m)
    if b == 0.0 or not np.isfinite(b):
        b = np.float32(1e-30)
    for _ in range(4):
        mant = int(np.abs(b).view(np.uint32) & 0x7FFFFF) | 0x800000
        if mant % 3 != 0:
            return float(b)
        b = np.nextafter(b, np.float32(np.sign(b) * np.float32(1e38)),
                         dtype=np.float32)
    return float(b)


def _build(weights, betas, dec_scale, dec_bias, y_bias):
    """Build the per-core SPMD Bass program.

    weights[i] = (D_i + K)/2 (stt scalar per level)
    betas[i]   = ACT bias for level i (threshold -beta/3)
    dec_scale  = 1/K, dec_bias = C/K      (symbol decode ts)
    y_bias     = C + c_0                  (value decode stt)
    """
    from contextlib import ExitStack

    import concourse.bass as bass
    import concourse.tile as tile
    from concourse import bacc, mybir

    f32 = mybir.dt.float32
    i32 = mybir.dt.int32
    Alu = mybir.AluOpType
    Act = mybir.ActivationFunctionType

    nc = bacc.Bacc(
        "TRN2",
        target_bir_lowering=False,
        debug=False,
        num_devices=N_CORES,
    )
    # row p = [x row | means row]: one DMA per tile feeds both halves
    xm = nc.dram_tensor("xm", [P, 2 * FREE], f32, kind="ExternalInput")
    xm_r = xm.rearrange("p (h q) -> p h q", h=2)
    # per-partition replicated constants: column i holds betas[i]
    nmid = nc.dram_tensor("nmid", [P, L], f32, kind="ExternalInput")
    i8 = mybir.dt.int8
    sym_out = nc.dram_tensor("sym", [P, FREE], i8 if SYM_I8 else i32,
                             kind="ExternalOutput")
    y_out = nc.dram_tensor("y", [P, FREE], f32, kind="ExternalOutput")

    S = Z_SPLIT
    with tile.TileContext(nc) as tc, ExitStack() as ctx:
        inp = ctx.enter_context(tc.tile_pool(name="inp", bufs=INP_BUFS))
        work = ctx.enter_context(tc.tile_pool(name="work", bufs=1))
        sgn = ctx.enter_context(tc.tile_pool(name="sgn", bufs=SGN_BUFS))
        outp = ctx.enter_context(tc.tile_pool(name="outp", bufs=OUTP_BUFS))
        cst = ctx.enter_context(tc.tile_pool(name="cst", bufs=1))

        nmt = cst.tile([P, L], f32, tag="nmt")
        nc.sync.dma_start(nmt[:], nmid[:])

        steps = REPEAT * N_TILES

        def emit_load_sub(k):
            # load tile k's interleaved input and compute r = x - means;
            # called mid-way through tile k-1's chain so the scalar engine
            # can pre-generate tile k's sign planes (kills the tile-boundary
            # bubble where DVE would wait on the first signs).
            sl_k = bass.ts(k % N_TILES, TILE_F)
            txm = inp.tile([P, 2 * TILE_F], f32, tag="txm", name=f"txm_{k}")
            nc.sync.dma_start(
                txm[:].rearrange("p (h f) -> p h f", h=2), xm_r[:, :, sl_k]
            )
            r = work.tile([P, TILE_F], f32, tag=f"r{k % 2}", name=f"r_{k}")
            nc.vector.tensor_sub(r[:], txm[:, :TILE_F], txm[:, TILE_F:])
            return txm, r

        nxt = emit_load_sub(0)
        for k in range(steps):
            t = k % N_TILES
            sl = bass.ts(t, TILE_F)
            txm, r = nxt
            tm = txm[:, TILE_F:]

            # packed accumulator, S independent in-place chains:
            #   z_c += W_i * sign(3r + beta_i)   (levels round-robin)
            zs = [
                work.tile([P, TILE_F], f32, tag=f"z{c}{k % 2}",
                          name=f"z{c}_{k}")
                for c in range(S)
            ]
            for i in range(L - 1):
                si = sgn.tile([P, TILE_F], f32, tag="s")
                nc.scalar.activation(si[:], r[:], Act.Sign,
                                     bias=nmt[:, i:i + 1], scale=3.0)
                z = zs[i % S]
                if i < S:
                    if ACT_INIT:
                        nc.scalar.activation(z[:], si[:], Act.Copy,
                                             scale=weights[i])
                    else:
                        nc.vector.tensor_scalar(z[:], si[:], weights[i],
                                                None, op0=Alu.mult)
                else:
                    nc.vector.scalar_tensor_tensor(
                        z[:], si[:], weights[i], z[:],
                        op0=Alu.mult, op1=Alu.add,
                    )
                if i == PIPE_MID and k + 1 < steps:
                    nxt = emit_load_sub(k + 1)
            for c in range(1, S):
                nc.vector.tensor_add(zs[0][:], zs[0][:], zs[c][:])
            cur = zs[0]

            # decode: sym = round(z/K + C/K)  (convert rounds to nearest)
            syi = outp.tile([P, TILE_F], i8 if SYM_I8 else i32, tag="syi")
            if ACT_DECODE:
                nc.scalar.activation(syi[:], cur[:], Act.Copy,
                                     bias=float(dec_bias), scale=dec_scale)
            else:
                nc.vector.tensor_scalar(syi[:], cur[:], dec_scale, dec_bias,
                                        op0=Alu.mult, op1=Alu.add)
            nc.sync.dma_start(sym_out[:, sl], syi[:])

            # y_hat = (z - K*symf) + (C + c0) + means
            sf = work.tile([P, TILE_F], f32, tag="sf")
            if ACT_DECODE:
                nc.scalar.activation(sf[:], syi[:], Act.Copy)
            else:
                nc.vector.tensor_scalar(sf[:], syi[:], 1.0, None,
                                        op0=Alu.mult)
            nc.vector.scalar_tensor_tensor(
                sf[:], sf[:], -K_ENC, cur[:], op0=Alu.mult, op1=Alu.add
            )
            yh = outp.tile([P, TILE_F], f32, tag="yh")
            nc.vector.scalar_tensor_tensor(
                yh[:], tm, y_bias, sf[:], op0=Alu.add, op1=Alu.add
            )
            nc.sync.dma_start(y_out[:, sl], yh[:])

    nc.compile()
    return nc


_cache = {}


def _get_nc(codebook):
    key = codebook.tobytes()
    if key not in _cache:
        cb = codebook.astype(np.float64)
        mids = ((cb[:-1] + cb[1:]) * 0.5).astype(np.float32).astype(np.float64)
        deltas = (cb[1:] - cb[:-1]).astype(np.float64)
        weights = [float(np.float32((d + K_ENC) * 0.5)) for d in deltas]
        betas = [_coprime3_beta(m) for m in mids]
        const = float(sum(np.float64(w) for w in weights))
        dec_scale = float(np.float32(1.0 / K_ENC))
        dec_bias = float(np.float32(const / K_ENC))
        y_bias = float(np.float32(const + cb[0]))
        nmid = np.zeros((P, L), np.float32)
        nmid[:, : L - 1] = np.float32(betas)[None, :]
        nc = _build(weights, betas, dec_scale, dec_bias, y_bias)
        _cache[key] = (nc, nmid)
    return _cache[key]


def _run(x, means, codebook, trace=False):
    from concourse.bass_utils import run_bass_kernel_spmd

    nc, nmid = _get_nc(np.asarray(codebook))

    x = np.asarray(x).reshape(N_CORES, P, FREE)
    means = np.asarray(means).reshape(N_CORES, P, FREE)
    in_maps = [
        {
            "xm": np.ascontiguousarray(np.concatenate([x[c], means[c]], axis=1)),
            "nmid": nmid,
        }
        for c in range(N_CORES)
    ]
    res = run_bass_kernel_spmd(
        nc, in_maps, core_ids=list(range(N_CORES)), trace=trace
    )
    sym = np.stack([res.results[c]["sym"] for c in range(N_CORES)])
    y = np.stack([res.results[c]["y"] for c in range(N_CORES)])
    sym = sym.reshape(B, C, H, W).astype(np.int32)
    y = y.reshape(B, C, H, W).astype(np.float32)
    return (sym, y), res


def kernel(x, means, codebook):
    (sym, y), _ = _run(x, means, codebook)
    return sym, y
